# revision 2
# baseline (speedup 1.0000x reference)
"""GQA kernel v7 for 8 Trainium2 NeuronCores.

Sharding: batch (2) x kv-group-pairs (4). Core c owns batch c//4 and
groups {2*(c%4), 2*(c%4)+1}. Host sums 4 fp16 partial out-projections
per batch.

Structure (on top of v5/v6):
  - lag-1 software pipeline over (qt, mt) units: scores+exp of unit i
    interleave with PV matmuls of unit i-1 (reading SBUF e tiles).
  - filler queue: k/v/q projections and out-projections are broken into
    1-4us pieces and drained one per pipeline step, so attention starts
    ~15us into the kernel and the PE queue stays dense (keeps the HAM
    clock gate open) without bulk-starving the ACT engine.
  - mask folded into V + masked-ones denominator column (exact); exp is
    bias-free, fused over (2 parities) x 512 queries per ACT op.
  - head-parity-packed layouts: score matmuls for the two parities hit
    disjoint PE row groups and run concurrently in the array.
  - denominators evacuated to 4 partitions x 2 free slots; two
    [128,1,512] reciprocals per q chunk (first half runs early);
    broadcast via selection-matrix matmuls.
"""

from collections import deque

import numpy as np

B = 2
S = 2048
H = 2048
NCORES = 8
QF = 512
KF = 128
D = 64
P = 128
SQ = 512
NCH = S // SQ
NKB = S // P
NHT = H // P
SCALE = 1.0 / np.sqrt(np.float32(D))


def _build_bass():
    import concourse.tile as tile
    from concourse import bacc, mybir

    f32 = mybir.dt.float32
    f32r = mybir.dt.float32r
    bf16 = mybir.dt.bfloat16
    f16 = mybir.dt.float16
    Exp = mybir.ActivationFunctionType.Exp

    nc = bacc.Bacc("TRN2", target_bir_lowering=False, debug=False)

    x4 = nc.dram_tensor("x4", [NCH, P, NHT, SQ], bf16, kind="ExternalInput").ap()
    wq = nc.dram_tensor("wq", [P, NHT, QF], bf16, kind="ExternalInput").ap()
    wk = nc.dram_tensor("wk", [P, NHT, KF], bf16, kind="ExternalInput").ap()
    wv = nc.dram_tensor("wv", [P, NHT, KF], bf16, kind="ExternalInput").ap()
    wo = nc.dram_tensor("wo", [P, QF // P, H], bf16, kind="ExternalInput").ap()
    mk = nc.dram_tensor("mk", [P, NKB], f32, kind="ExternalInput").ap()
    selin = nc.dram_tensor("selin", [P, 8, D], f32r, kind="ExternalInput").ap()
    outT = nc.dram_tensor("outT", [P, NHT, S], f16, kind="ExternalOutput").ap()

    with tile.TileContext(nc) as tc:
        with (
            nc.allow_low_precision(reason="bf16 matmuls, fp16 partial outputs"),
            tc.tile_pool(name="const", bufs=1) as const_pool,
            tc.tile_pool(name="w", bufs=1) as w_pool,
            tc.tile_pool(name="xc", bufs=3) as xc_pool,
            tc.tile_pool(name="qt", bufs=1) as qt_pool,
            tc.tile_pool(name="kt", bufs=1) as kt_pool,
            tc.tile_pool(name="vs", bufs=1) as v_pool,
            tc.tile_pool(name="at", bufs=2) as at_pool,
            tc.tile_pool(name="e", bufs=20) as e_pool,
            tc.tile_pool(name="dn", bufs=2) as dn_pool,
            tc.tile_pool(name="dnr", bufs=2) as dnr_pool,
            tc.tile_pool(name="rb", bufs=2) as rb_pool,
            tc.tile_pool(name="o", bufs=3) as o_pool,
            # PSUM banks: pss 2x2 + po 2 + fill 2x1 = 8
            tc.tile_pool(name="ps4", bufs=2, space="PSUM") as ps4_pool,
            tc.tile_pool(name="pso", bufs=2, space="PSUM") as pso_pool,
            tc.tile_pool(name="psb", bufs=1, space="PSUM") as psb_pool,
            tc.tile_pool(name="psc", bufs=1, space="PSUM") as psc_pool,
        ):
            # constants
            mk_sb = const_pool.tile([P, NKB], f32, tag="mk")
            nc.sync.dma_start(out=mk_sb, in_=mk)
            sel = const_pool.tile([P, 8, D], f32r, tag="sel")
            nc.sync.dma_start(out=sel, in_=selin)

            # weights
            wq_sb = w_pool.tile([P, NHT, QF], bf16, tag="wq")
            wk_sb = w_pool.tile([P, NHT, KF], bf16, tag="wk")
            wv_sb = w_pool.tile([P, NHT, KF], bf16, tag="wv")
            wo_sb = w_pool.tile([P, QF // P, H], bf16, tag="wo")
            nc.sync.dma_start(out=wk_sb, in_=wk)
            nc.sync.dma_start(out=wv_sb, in_=wv)
            nc.sync.dma_start(out=wq_sb, in_=wq)
            nc.sync.dma_start(out=wo_sb, in_=wo)

            qt_sb = qt_pool.tile([P, QF // P, S], bf16, tag="qt")
            kt_sb = kt_pool.tile([P, S], bf16, tag="kt")
            v_sb = v_pool.tile([P, NKB, 2, D + 1], bf16, tag="v")

            # two 1-bank PSUM slots for filler pieces, round-robin
            _fill_pools = [psb_pool, psc_pool]
            _fill_toggle = [0]

            def fill_psum():
                pool = _fill_pools[_fill_toggle[0] & 1]
                _fill_toggle[0] += 1
                return pool.tile([P, SQ], f32, tag="psf", name="psf")

            xkv = {}
            xq = {}

            def k_chunk(c):
                s0 = c * SQ
                xc = xc_pool.tile([P, NHT, SQ], bf16, tag="xc", name="xkv")
                nc.sync.dma_start(out=xc, in_=x4[c])
                xkv[c] = xc
                psk = fill_psum()
                for ht in range(NHT):
                    nc.tensor.matmul(
                        psk, lhsT=wk_sb[:, ht, :], rhs=xc[:, ht, :],
                        start=(ht == 0), stop=(ht == NHT - 1),
                    )
                nc.vector.tensor_copy(kt_sb[:, s0:s0 + SQ], psk)

            def v_piece(c, st):
                xc = xkv[c]
                kb = c * (SQ // P) + st
                psv = fill_psum()
                for ht in range(NHT):
                    nc.tensor.matmul(
                        psv[:, 0:KF],
                        lhsT=xc[:, ht, st * P:(st + 1) * P],
                        rhs=wv_sb[:, ht, :],
                        start=(ht == 0), stop=(ht == NHT - 1),
                    )
                nc.vector.tensor_scalar_mul(
                    v_sb[:, kb, :, 0:D], psv[:, 0:KF], mk_sb[:, kb:kb + 1]
                )
                nc.vector.tensor_copy(v_sb[:, kb, 0, D:D + 1],
                                      mk_sb[:, kb:kb + 1])
                nc.vector.tensor_copy(v_sb[:, kb, 1, D:D + 1],
                                      mk_sb[:, kb:kb + 1])

            def qp_chunk(qt, mt):
                s0 = qt * SQ
                if mt == 0:
                    xc = xc_pool.tile([P, NHT, SQ], bf16, tag="xc", name="xq")
                    nc.sync.dma_start(out=xc, in_=x4[qt])
                    xq[qt] = xc
                xc = xq[qt]
                psq = fill_psum()
                for ht in range(NHT):
                    nc.tensor.matmul(
                        psq,
                        lhsT=wq_sb[:, ht, mt * P:(mt + 1) * P],
                        rhs=xc[:, ht, :],
                        start=(ht == 0), stop=(ht == NHT - 1),
                    )
                nc.vector.tensor_copy(qt_sb[:, mt, s0:s0 + SQ], psq)

            def out_block(qt, mt, at_sb):
                q0 = qt * SQ
                psC = fill_psum()
                for kb4 in range(QF // P):
                    nc.tensor.matmul(
                        psC,
                        lhsT=wo_sb[:, kb4, mt * P:(mt + 1) * P],
                        rhs=at_sb[:, kb4, :],
                        start=(kb4 == 0), stop=(kb4 == QF // P - 1),
                    )
                o = o_pool.tile([P, SQ], f16, tag="o")
                nc.vector.tensor_copy(o, psC)
                nc.sync.dma_start(out=outT[:, mt, q0:q0 + SQ], in_=o)

            # ---- attention pipeline pieces ----
            def scores_step(u, s, e_store):
                qt, mt = u
                q0 = qt * SQ
                kb = s
                pss = ps4_pool.tile([P, 2, SQ], f32, tag="ps4")
                for par in range(2):
                    r0 = par * D
                    nc.tensor.matmul(
                        pss[:, par, :],
                        lhsT=kt_sb[r0:r0 + D, kb * P:(kb + 1) * P],
                        rhs=qt_sb[r0:r0 + D, mt, q0:q0 + SQ],
                        start=True, stop=True,
                    )
                e2 = e_pool.tile([P, 2, SQ], bf16, tag="e")
                nc.scalar.activation(e2, pss, Exp, scale=float(SCALE))
                e_store.append(e2)

            def pv_step(prev, s, e_store, po_pair):
                po_a, po_b = po_pair
                e2 = e_store[s]
                kb = s
                nc.tensor.matmul(
                    po_a, lhsT=v_sb[:, kb, 0, :], rhs=e2[:, 0, :],
                    start=(kb == 0), stop=(kb == NKB - 1),
                )
                nc.tensor.matmul(
                    po_b, lhsT=v_sb[:, kb, 1, :], rhs=e2[:, 1, :],
                    start=(kb == 0), stop=(kb == NKB - 1),
                )

            def evac(prev, po_pair, at_sb, dn):
                _, mt = prev
                # dn lane: slot = mt//2, partition 32*(2*(mt%2)+par)
                for par, po in ((0, po_pair[0]), (1, po_pair[1])):
                    r0 = par * D
                    lane = 2 * (mt % 2) + par
                    nc.vector.tensor_copy(dn[32 * lane:32 * lane + 1,
                                             mt // 2, :],
                                          po[D:D + 1, :])
                    nc.vector.tensor_copy(at_sb[r0:r0 + D, mt, :], po[0:D, :])

            def normalize_half(qt, half, at_sb, dn, dnr):
                nc.vector.reciprocal(dnr[:, half, :], dn[:, half, :])
                for mt in (2 * half, 2 * half + 1):
                    rb = rb_pool.tile([P, SQ], f32, tag="rb")
                    for par in range(2):
                        h = 2 * mt + par
                        pb = psb_pool.tile([D, SQ], f32, tag="psf",
                                           name="pb")
                        nc.tensor.matmul(pb, lhsT=sel[:, h, :],
                                         rhs=dnr[:, mt // 2, :],
                                         start=True, stop=True)
                        nc.vector.tensor_copy(rb[par * D:(par + 1) * D, :], pb)
                    nc.vector.tensor_mul(
                        at_sb[:, mt, :], at_sb[:, mt, :], rb,
                    )

            # ---- emission engine ----
            fillers = deque()
            units = [(qt, mt) for qt in range(NCH) for mt in range(QF // P)]
            e_map = {}
            po_map = {}
            at_map = {}
            dn_map = {}
            dnr_map = {}

            # Minimal pre-work before the first score matmul; the rest of the
            # projections drain through the filler queue, one piece per
            # pipeline step. Order respects emission-order deadlines:
            # kt chunk c is read by unit (0,0) step 4c; v chunk c by unit
            # (0,1) step 4c; qp(0,mt) by unit (0,mt) step 0.
            k_chunk(0)
            qp_chunk(0, 0)
            fillers.append(lambda: k_chunk(1))      # drained @ step 1 (<4)
            for st in range(4):                     # @2-5 (<16)
                fillers.append((lambda st=st: v_piece(0, st)))
            fillers.append(lambda: k_chunk(2))      # @6 (<8)
            for st in range(4):                     # @7-10 (<20)
                fillers.append((lambda st=st: v_piece(1, st)))
            fillers.append(lambda: k_chunk(3))      # @11 (<12)
            fillers.append(lambda: qp_chunk(0, 1))  # @12 (<16)
            for st in range(4):                     # @13-16 (<24)
                fillers.append((lambda st=st: v_piece(2, st)))
            fillers.append(lambda: qp_chunk(0, 2))  # @17 (<32)
            for st in range(4):                     # @18-21 (<28)
                fillers.append((lambda st=st: v_piece(3, st)))
            fillers.append(lambda: qp_chunk(0, 3))  # @22 (<48)

            for idx in range(len(units) + 1):
                u = units[idx] if idx < len(units) else None
                prev = units[idx - 1] if idx >= 1 else None
                if u is not None:
                    qt, mt = u
                    if mt == 0:
                        at_map[qt] = at_pool.tile([P, QF // P, SQ], bf16,
                                                  tag="at", name="at_sb")
                        dn = dn_pool.tile([P, 2, SQ], f32, tag="dn")
                        nc.vector.memset(dn, 1.0)
                        dn_map[qt] = dn
                        dnr_map[qt] = dnr_pool.tile([P, 2, SQ], f32r,
                                                    tag="dnr", name="dnr")
                    if mt == 1 and qt + 1 < NCH:
                        for m2 in range(4):
                            fillers.append(
                                (lambda q=qt + 1, m=m2: qp_chunk(q, m)))
                    if mt == 3:
                        # first-half normalize: units (qt,0),(qt,1) evac'd
                        fillers.append(
                            (lambda q=qt: normalize_half(
                                q, 0, at_map[q], dn_map[q], dnr_map[q])))
                    e_map[u] = []
                if prev is not None:
                    po_map[prev] = (
                        pso_pool.tile([D + 1, SQ], f32, tag="pso", name="po_a"),
                        pso_pool.tile([D + 1, SQ], f32, tag="pso", name="po_b"),
                    )
                for s in range(NKB):
                    if u is not None:
                        scores_step(u, s, e_map[u])
                    if prev is not None:
                        pv_step(prev, s, e_map[prev], po_map[prev])
                    if fillers:
                        fillers.popleft()()
                if prev is not None:
                    pqt, pmt = prev
                    evac(prev, po_map[prev], at_map[pqt], dn_map[pqt])
                    del e_map[prev]
                    del po_map[prev]
                    if pmt == QF // P - 1:
                        # second-half normalize + queue out-projections
                        def _fin(q=pqt):
                            normalize_half(q, 1, at_map[q], dn_map[q],
                                           dnr_map[q])
                        fillers.appendleft(_fin)
                        def _mk_out(q, m):
                            return lambda: out_block(q, m, at_map[q])
                        for m in range(NHT):
                            fillers.append(_mk_out(pqt, m))
            while fillers:
                fillers.popleft()()
    nc.compile()
    return nc


_NC_CACHE = None


def _get_nc():
    global _NC_CACHE
    if _NC_CACHE is None:
        _NC_CACHE = _build_bass()
    return _NC_CACHE


def _make_in_maps(inputs):
    import ml_dtypes

    bf16 = ml_dtypes.bfloat16
    x = np.asarray(inputs["x"], dtype=np.float32)
    mask = np.asarray(inputs["mask"])
    Wq = np.asarray(inputs["Wq"], dtype=np.float32)
    Wk = np.asarray(inputs["Wk"], dtype=np.float32)
    Wv = np.asarray(inputs["Wv"], dtype=np.float32)
    Wo = np.asarray(inputs["Wo"], dtype=np.float32)

    x4s = []
    for b in range(B):
        xt = x[b].T
        x4b = np.ascontiguousarray(
            xt.reshape(NHT, P, NCH, SQ).transpose(2, 1, 0, 3)
        ).astype(bf16)
        x4s.append(x4b)
    mks = []
    for b in range(B):
        m = mask[b, 0, 0, 0, :].astype(np.float32)
        mks.append(np.ascontiguousarray(m.reshape(NKB, P).T))

    in_maps = []
    for cid in range(NCORES):
        b, gp = divmod(cid, 4)
        g0, g1 = 2 * gp, 2 * gp + 1
        qrows = np.empty(QF, dtype=np.int64)
        for mt in range(4):
            for par in range(2):
                g = 2 * gp + par
                h = g * 4 + mt
                qrows[mt * P + par * D:mt * P + par * D + D] = np.arange(
                    h * D, (h + 1) * D
                )
        krows = np.concatenate([np.arange(g0 * D, (g0 + 1) * D),
                                np.arange(g1 * D, (g1 + 1) * D)])
        wq_h = np.ascontiguousarray(
            Wq[qrows, :].T.reshape(NHT, P, QF).transpose(1, 0, 2)
        ).astype(bf16)
        wk_h = np.ascontiguousarray(
            Wk[krows, :].T.reshape(NHT, P, KF).transpose(1, 0, 2)
        ).astype(bf16)
        wv_h = np.ascontiguousarray(
            Wv[krows, :].T.reshape(NHT, P, KF).transpose(1, 0, 2)
        ).astype(bf16)
        colidx = np.empty((P, QF // P), dtype=np.int64)
        for p in range(P):
            par, dd = divmod(p, D)
            for t4 in range(QF // P):
                colidx[p, t4] = ((2 * gp + par) * 4 + t4) * D + dd
        wo_h = np.ascontiguousarray(
            Wo[:, colidx].transpose(1, 2, 0)
        ).astype(bf16)
        # sel[p, h, :] = 1 iff p == 32*(2*(mt%2)+par), h = 2*mt+par
        sel_h = np.zeros((P, 8, D), dtype=np.float32)
        for h in range(8):
            mt, par = divmod(h, 2)
            sel_h[32 * (2 * (mt % 2) + par), h, :] = 1.0
        in_maps.append({
            "x4": x4s[b],
            "wq": wq_h,
            "wk": wk_h,
            "wv": wv_h,
            "wo": wo_h,
            "mk": mks[b],
            "selin": sel_h,
        })
    return in_maps


def kernel(**inputs):
    from concourse.bass_utils import run_bass_kernel_spmd

    in_maps = _make_in_maps(inputs)
    nc = _get_nc()
    res = run_bass_kernel_spmd(nc, in_maps, core_ids=list(range(NCORES)))
    outs = [r["outT"] for r in res.results]
    out = np.empty((B, S, H), dtype=np.float32)
    for b in range(B):
        acc = np.zeros((H, S), dtype=np.float32)
        for c in range(4 * b, 4 * b + 4):
            full = outs[c].transpose(1, 0, 2).reshape(H, S).astype(np.float32)
            acc += full
        out[b] = acc.T
    return out


# revision 3
# speedup vs baseline: 1.0309x; 1.0309x over previous
"""GQA kernel v7 for 8 Trainium2 NeuronCores.

Sharding: batch (2) x kv-group-pairs (4). Core c owns batch c//4 and
groups {2*(c%4), 2*(c%4)+1}. Host sums 4 fp16 partial out-projections
per batch.

Structure (on top of v5/v6):
  - lag-1 software pipeline over (qt, mt) units: scores+exp of unit i
    interleave with PV matmuls of unit i-1 (reading SBUF e tiles).
  - filler queue: k/v/q projections and out-projections are broken into
    1-4us pieces and drained one per pipeline step, so attention starts
    ~15us into the kernel and the PE queue stays dense (keeps the HAM
    clock gate open) without bulk-starving the ACT engine.
  - mask folded into V + masked-ones denominator column (exact); exp is
    bias-free, fused over (2 parities) x 512 queries per ACT op.
  - head-parity-packed layouts: score matmuls for the two parities hit
    disjoint PE row groups and run concurrently in the array.
  - denominators evacuated to 4 partitions x 2 free slots; two
    [128,1,512] reciprocals per q chunk (first half runs early);
    broadcast via selection-matrix matmuls.
"""

from collections import deque

import numpy as np

B = 2
S = 2048
H = 2048
NCORES = 8
QF = 512
KF = 128
D = 64
P = 128
SQ = 512
NCH = S // SQ
NKB = S // P
NHT = H // P
SCALE = 1.0 / np.sqrt(np.float32(D))


def _build_bass():
    import concourse.tile as tile
    from concourse import bacc, mybir

    f32 = mybir.dt.float32
    f32r = mybir.dt.float32r
    bf16 = mybir.dt.bfloat16
    f16 = mybir.dt.float16
    Exp = mybir.ActivationFunctionType.Exp

    nc = bacc.Bacc("TRN2", target_bir_lowering=False, debug=False)

    x4 = nc.dram_tensor("x4", [NCH, P, NHT, SQ], bf16, kind="ExternalInput").ap()
    wq = nc.dram_tensor("wq", [P, NHT, QF], bf16, kind="ExternalInput").ap()
    wk = nc.dram_tensor("wk", [P, NHT, KF], bf16, kind="ExternalInput").ap()
    wv = nc.dram_tensor("wv", [P, NHT, KF], bf16, kind="ExternalInput").ap()
    wo = nc.dram_tensor("wo", [P, QF // P, H], bf16, kind="ExternalInput").ap()
    mk = nc.dram_tensor("mk", [P, NKB], f32, kind="ExternalInput").ap()
    selin = nc.dram_tensor("selin", [P, 8, D], f32r, kind="ExternalInput").ap()
    outT = nc.dram_tensor("outT", [P, NHT, S], f16, kind="ExternalOutput").ap()

    with tile.TileContext(nc) as tc:
        with (
            nc.allow_low_precision(reason="bf16 matmuls, fp16 partial outputs"),
            tc.tile_pool(name="const", bufs=1) as const_pool,
            tc.tile_pool(name="w", bufs=1) as w_pool,
            tc.tile_pool(name="xc", bufs=3) as xc_pool,
            tc.tile_pool(name="qt", bufs=1) as qt_pool,
            tc.tile_pool(name="kt", bufs=1) as kt_pool,
            tc.tile_pool(name="vs", bufs=1) as v_pool,
            tc.tile_pool(name="at", bufs=2) as at_pool,
            tc.tile_pool(name="e", bufs=20) as e_pool,
            tc.tile_pool(name="dn", bufs=2) as dn_pool,
            tc.tile_pool(name="dnr", bufs=2) as dnr_pool,
            tc.tile_pool(name="rb", bufs=2) as rb_pool,
            tc.tile_pool(name="stg", bufs=4) as stg_pool,
            tc.tile_pool(name="o", bufs=3) as o_pool,
            # PSUM banks: pss 2x2 + po 2 + fill 2x1 = 8
            tc.tile_pool(name="ps4", bufs=2, space="PSUM") as ps4_pool,
            tc.tile_pool(name="pso", bufs=2, space="PSUM") as pso_pool,
            tc.tile_pool(name="psb", bufs=1, space="PSUM") as psb_pool,
            tc.tile_pool(name="psc", bufs=1, space="PSUM") as psc_pool,
        ):
            # constants
            mk_sb = const_pool.tile([P, NKB], f32, tag="mk")
            nc.sync.dma_start(out=mk_sb, in_=mk)
            sel = const_pool.tile([P, 8, D], f32r, tag="sel")
            nc.sync.dma_start(out=sel, in_=selin)

            # weights
            wq_sb = w_pool.tile([P, NHT, QF], bf16, tag="wq")
            wk_sb = w_pool.tile([P, NHT, KF], bf16, tag="wk")
            wv_sb = w_pool.tile([P, NHT, KF], bf16, tag="wv")
            wo_sb = w_pool.tile([P, QF // P, H], bf16, tag="wo")
            nc.sync.dma_start(out=wk_sb, in_=wk)
            nc.sync.dma_start(out=wv_sb, in_=wv)
            nc.sync.dma_start(out=wq_sb, in_=wq)
            nc.sync.dma_start(out=wo_sb, in_=wo)

            # warm-up matmuls on the early-arriving sel constant: keeps the
            # PE HAM busy through the initial input-DMA wait so the first
            # projections run at full clock. Output is discarded.
            pswu = psb_pool.tile([D, SQ], f32, tag="psf", name="pswu")
            for _ in range(64):
                nc.tensor.matmul(pswu, lhsT=sel[:, 0, :],
                                 rhs=sel.rearrange("p a b -> p (a b)"),
                                 start=True, stop=True)

            qt_sb = qt_pool.tile([P, QF // P, S], bf16, tag="qt")
            kt_sb = kt_pool.tile([P, S], bf16, tag="kt")
            v_sb = v_pool.tile([P, NKB, 2, D + 1], bf16, tag="v")

            # two 1-bank PSUM slots for filler pieces, round-robin
            _fill_pools = [psb_pool, psc_pool]
            _fill_toggle = [0]

            def fill_psum():
                pool = _fill_pools[_fill_toggle[0] & 1]
                _fill_toggle[0] += 1
                return pool.tile([P, SQ], f32, tag="psf", name="psf")

            xkv = {}
            xq = {}

            def k_chunk(c):
                s0 = c * SQ
                xc = xc_pool.tile([P, NHT, SQ], bf16, tag="xc", name="xkv")
                nc.sync.dma_start(out=xc, in_=x4[c])
                xkv[c] = xc
                psk = fill_psum()
                for ht in range(NHT):
                    nc.tensor.matmul(
                        psk, lhsT=wk_sb[:, ht, :], rhs=xc[:, ht, :],
                        start=(ht == 0), stop=(ht == NHT - 1),
                    )
                nc.vector.tensor_copy(kt_sb[:, s0:s0 + SQ], psk)

            def v_piece(c, st):
                xc = xkv[c]
                kb = c * (SQ // P) + st
                psv = fill_psum()
                for ht in range(NHT):
                    nc.tensor.matmul(
                        psv[:, 0:KF],
                        lhsT=xc[:, ht, st * P:(st + 1) * P],
                        rhs=wv_sb[:, ht, :],
                        start=(ht == 0), stop=(ht == NHT - 1),
                    )
                nc.vector.tensor_scalar_mul(
                    v_sb[:, kb, :, 0:D], psv[:, 0:KF], mk_sb[:, kb:kb + 1]
                )
                nc.vector.tensor_copy(v_sb[:, kb, 0, D:D + 1],
                                      mk_sb[:, kb:kb + 1])
                nc.vector.tensor_copy(v_sb[:, kb, 1, D:D + 1],
                                      mk_sb[:, kb:kb + 1])

            def qp_chunk(qt, mt):
                s0 = qt * SQ
                if mt == 0:
                    xc = xc_pool.tile([P, NHT, SQ], bf16, tag="xc", name="xq")
                    nc.sync.dma_start(out=xc, in_=x4[qt])
                    xq[qt] = xc
                xc = xq[qt]
                psq = fill_psum()
                for ht in range(NHT):
                    nc.tensor.matmul(
                        psq,
                        lhsT=wq_sb[:, ht, mt * P:(mt + 1) * P],
                        rhs=xc[:, ht, :],
                        start=(ht == 0), stop=(ht == NHT - 1),
                    )
                nc.vector.tensor_copy(qt_sb[:, mt, s0:s0 + SQ], psq)

            def out_block(qt, mt, at_sb):
                q0 = qt * SQ
                psC = fill_psum()
                for kb4 in range(QF // P):
                    nc.tensor.matmul(
                        psC,
                        lhsT=wo_sb[:, kb4, mt * P:(mt + 1) * P],
                        rhs=at_sb[:, kb4, :],
                        start=(kb4 == 0), stop=(kb4 == QF // P - 1),
                    )
                o = o_pool.tile([P, SQ], f16, tag="o")
                nc.vector.tensor_copy(o, psC)
                nc.sync.dma_start(out=outT[:, mt, q0:q0 + SQ], in_=o)

            # ---- attention pipeline pieces ----
            def scores_step(u, s, e_store):
                qt, mt = u
                q0 = qt * SQ
                kb = s
                pss = ps4_pool.tile([P, 2, SQ], f32, tag="ps4")
                for par in range(2):
                    r0 = par * D
                    nc.tensor.matmul(
                        pss[:, par, :],
                        lhsT=kt_sb[r0:r0 + D, kb * P:(kb + 1) * P],
                        rhs=qt_sb[r0:r0 + D, mt, q0:q0 + SQ],
                        start=True, stop=True,
                    )
                e2 = e_pool.tile([P, 2, SQ], bf16, tag="e")
                nc.scalar.activation(e2, pss, Exp, scale=float(SCALE))
                e_store.append(e2)

            def pv_step(prev, s, e_store, po_pair):
                po_a, po_b = po_pair
                e2 = e_store[s]
                kb = s
                nc.tensor.matmul(
                    po_a, lhsT=v_sb[:, kb, 0, :], rhs=e2[:, 0, :],
                    start=(kb == 0), stop=(kb == NKB - 1),
                )
                nc.tensor.matmul(
                    po_b, lhsT=v_sb[:, kb, 1, :], rhs=e2[:, 1, :],
                    start=(kb == 0), stop=(kb == NKB - 1),
                )

            def evac(prev, po_pair, at_sb, dn):
                _, mt = prev
                # single [65,512] copy per parity frees the po PSUM pair in
                # ~0.8us; numerator/denominator redistribute from SBUF after
                # (off the PV critical path). dn lane: slot = mt//2,
                # partition 32*(2*(mt%2)+par).
                stgs = []
                for par, po in ((0, po_pair[0]), (1, po_pair[1])):
                    stg = stg_pool.tile([D + 1, SQ], bf16, tag="stg",
                                        name="stg")
                    nc.vector.tensor_copy(stg, po)
                    stgs.append(stg)
                for par, stg in ((0, stgs[0]), (1, stgs[1])):
                    r0 = par * D
                    lane = 2 * (mt % 2) + par
                    nc.vector.tensor_copy(at_sb[r0:r0 + D, mt, :], stg[0:D, :])
                    nc.vector.tensor_copy(dn[32 * lane:32 * lane + 1,
                                             mt // 2, :],
                                          stg[D:D + 1, :])

            def normalize_half(qt, half, at_sb, dn, dnr):
                nc.vector.reciprocal(dnr[:, half, :], dn[:, half, :])
                for mt in (2 * half, 2 * half + 1):
                    rb = rb_pool.tile([P, SQ], f32, tag="rb")
                    for par in range(2):
                        h = 2 * mt + par
                        pb = psb_pool.tile([D, SQ], f32, tag="psf",
                                           name="pb")
                        nc.tensor.matmul(pb, lhsT=sel[:, h, :],
                                         rhs=dnr[:, mt // 2, :],
                                         start=True, stop=True)
                        nc.vector.tensor_copy(rb[par * D:(par + 1) * D, :], pb)
                    nc.vector.tensor_mul(
                        at_sb[:, mt, :], at_sb[:, mt, :], rb,
                    )

            # ---- emission engine ----
            fillers = deque()
            units = [(qt, mt) for qt in range(NCH) for mt in range(QF // P)]
            e_map = {}
            po_map = {}
            at_map = {}
            dn_map = {}
            dnr_map = {}

            # Minimal pre-work before the first score matmul; the rest of the
            # projections drain through the filler queue, one piece per
            # pipeline step. Order respects emission-order deadlines:
            # kt chunk c is read by unit (0,0) step 4c; v chunk c by unit
            # (0,1) step 4c; qp(0,mt) by unit (0,mt) step 0.
            k_chunk(0)
            qp_chunk(0, 0)
            fillers.append(lambda: k_chunk(1))      # drained @ step 1 (<4)
            for st in range(4):                     # @2-5 (<16)
                fillers.append((lambda st=st: v_piece(0, st)))
            fillers.append(lambda: k_chunk(2))      # @6 (<8)
            for st in range(4):                     # @7-10 (<20)
                fillers.append((lambda st=st: v_piece(1, st)))
            fillers.append(lambda: k_chunk(3))      # @11 (<12)
            fillers.append(lambda: qp_chunk(0, 1))  # @12 (<16)
            for st in range(4):                     # @13-16 (<24)
                fillers.append((lambda st=st: v_piece(2, st)))
            fillers.append(lambda: qp_chunk(0, 2))  # @17 (<32)
            for st in range(4):                     # @18-21 (<28)
                fillers.append((lambda st=st: v_piece(3, st)))
            fillers.append(lambda: qp_chunk(0, 3))  # @22 (<48)

            for idx in range(len(units) + 1):
                u = units[idx] if idx < len(units) else None
                prev = units[idx - 1] if idx >= 1 else None
                if u is not None:
                    qt, mt = u
                    if mt == 0:
                        at_map[qt] = at_pool.tile([P, QF // P, SQ], bf16,
                                                  tag="at", name="at_sb")
                        dn = dn_pool.tile([P, 2, SQ], f32, tag="dn")
                        nc.vector.memset(dn, 1.0)
                        dn_map[qt] = dn
                        dnr_map[qt] = dnr_pool.tile([P, 2, SQ], f32r,
                                                    tag="dnr", name="dnr")
                    if mt == 1 and qt + 1 < NCH:
                        for m2 in range(4):
                            fillers.append(
                                (lambda q=qt + 1, m=m2: qp_chunk(q, m)))
                    if mt == 3:
                        # first-half normalize: units (qt,0),(qt,1) evac'd
                        fillers.append(
                            (lambda q=qt: normalize_half(
                                q, 0, at_map[q], dn_map[q], dnr_map[q])))
                    e_map[u] = []
                if prev is not None:
                    po_map[prev] = (
                        pso_pool.tile([D + 1, SQ], f32, tag="pso", name="po_a"),
                        pso_pool.tile([D + 1, SQ], f32, tag="pso", name="po_b"),
                    )
                for s in range(NKB):
                    if u is not None:
                        scores_step(u, s, e_map[u])
                    if prev is not None:
                        pv_step(prev, s, e_map[prev], po_map[prev])
                    if fillers:
                        fillers.popleft()()
                if prev is not None:
                    pqt, pmt = prev
                    evac(prev, po_map[prev], at_map[pqt], dn_map[pqt])
                    del e_map[prev]
                    del po_map[prev]
                    if pmt == QF // P - 1:
                        # second-half normalize + queue out-projections
                        def _fin(q=pqt):
                            normalize_half(q, 1, at_map[q], dn_map[q],
                                           dnr_map[q])
                        fillers.appendleft(_fin)
                        def _mk_out(q, m):
                            return lambda: out_block(q, m, at_map[q])
                        for m in range(NHT):
                            fillers.append(_mk_out(pqt, m))
            while fillers:
                fillers.popleft()()
    nc.compile()
    return nc


_NC_CACHE = None


def _get_nc():
    global _NC_CACHE
    if _NC_CACHE is None:
        _NC_CACHE = _build_bass()
    return _NC_CACHE


def _make_in_maps(inputs):
    import ml_dtypes

    bf16 = ml_dtypes.bfloat16
    x = np.asarray(inputs["x"], dtype=np.float32)
    mask = np.asarray(inputs["mask"])
    Wq = np.asarray(inputs["Wq"], dtype=np.float32)
    Wk = np.asarray(inputs["Wk"], dtype=np.float32)
    Wv = np.asarray(inputs["Wv"], dtype=np.float32)
    Wo = np.asarray(inputs["Wo"], dtype=np.float32)

    x4s = []
    for b in range(B):
        xt = x[b].T
        x4b = np.ascontiguousarray(
            xt.reshape(NHT, P, NCH, SQ).transpose(2, 1, 0, 3)
        ).astype(bf16)
        x4s.append(x4b)
    mks = []
    for b in range(B):
        m = mask[b, 0, 0, 0, :].astype(np.float32)
        mks.append(np.ascontiguousarray(m.reshape(NKB, P).T))

    in_maps = []
    for cid in range(NCORES):
        b, gp = divmod(cid, 4)
        g0, g1 = 2 * gp, 2 * gp + 1
        qrows = np.empty(QF, dtype=np.int64)
        for mt in range(4):
            for par in range(2):
                g = 2 * gp + par
                h = g * 4 + mt
                qrows[mt * P + par * D:mt * P + par * D + D] = np.arange(
                    h * D, (h + 1) * D
                )
        krows = np.concatenate([np.arange(g0 * D, (g0 + 1) * D),
                                np.arange(g1 * D, (g1 + 1) * D)])
        wq_h = np.ascontiguousarray(
            Wq[qrows, :].T.reshape(NHT, P, QF).transpose(1, 0, 2)
        ).astype(bf16)
        wk_h = np.ascontiguousarray(
            Wk[krows, :].T.reshape(NHT, P, KF).transpose(1, 0, 2)
        ).astype(bf16)
        wv_h = np.ascontiguousarray(
            Wv[krows, :].T.reshape(NHT, P, KF).transpose(1, 0, 2)
        ).astype(bf16)
        colidx = np.empty((P, QF // P), dtype=np.int64)
        for p in range(P):
            par, dd = divmod(p, D)
            for t4 in range(QF // P):
                colidx[p, t4] = ((2 * gp + par) * 4 + t4) * D + dd
        wo_h = np.ascontiguousarray(
            Wo[:, colidx].transpose(1, 2, 0)
        ).astype(bf16)
        # sel[p, h, :] = 1 iff p == 32*(2*(mt%2)+par), h = 2*mt+par
        sel_h = np.zeros((P, 8, D), dtype=np.float32)
        for h in range(8):
            mt, par = divmod(h, 2)
            sel_h[32 * (2 * (mt % 2) + par), h, :] = 1.0
        in_maps.append({
            "x4": x4s[b],
            "wq": wq_h,
            "wk": wk_h,
            "wv": wv_h,
            "wo": wo_h,
            "mk": mks[b],
            "selin": sel_h,
        })
    return in_maps


def kernel(**inputs):
    from concourse.bass_utils import run_bass_kernel_spmd

    in_maps = _make_in_maps(inputs)
    nc = _get_nc()
    res = run_bass_kernel_spmd(nc, in_maps, core_ids=list(range(NCORES)))
    outs = [r["outT"] for r in res.results]
    out = np.empty((B, S, H), dtype=np.float32)
    for b in range(B):
        acc = np.zeros((H, S), dtype=np.float32)
        for c in range(4 * b, 4 * b + 4):
            full = outs[c].transpose(1, 0, 2).reshape(H, S).astype(np.float32)
            acc += full
        out[b] = acc.T
    return out
